# revision 7
# baseline (speedup 1.0000x reference)
"""Additive (Bahdanau) attention kernel for one TRN2 chip (8 NeuronCores).

Computes, for query (B,D), keys (B,S,D), mask (B,S), W1 (A,D), W2 (A,D), v (A,):
    scores[b,s] = v . tanh(W1 @ query[b] + W2 @ keys[b,s])
    out = softmax(scores - 1e30 * ~mask, axis=-1)

Sharding: data-parallel over batch B across the 8 cores (4 batches/core);
W1/W2/v replicated. No collectives needed; per-core outputs are concatenated
on the host.

Per-core device kernel (all matmuls in float32r at full PE rate):
  - w1q[a,b]    = W1 @ q_b           (tiny matmul, a on partitions)
  - per (b, s-tile of 512), per a-block of 128:
        psum[a,s] += W2T_blk.T @ keysT_tile   (8 k-blocks)
        comb = tanh(psum + w1q[:,b])          (ScalarE, per-partition bias)
        sc_psum[0:4,s] += v_blk.T @ comb      (M=4 matmul, partition reduce)
  - scores[b,s] = sc_psum + maskadd           (additive -1e30 mask)
  - softmax tail on [4, 2048]: -max -> exp(+bias, accum sum) -> recip -> scale
"""

import numpy as np

B, S, D, A = 32, 2048, 1024, 1024
NCORES = 8
BL = B // NCORES  # 4 batches per core
ST = 512          # s-tile width
NST = S // ST     # 4 s-tiles per batch
KB = D // 128     # 8 contraction blocks
JB = A // 128     # 8 attn-dim blocks
MASK_NEG = 1e30

_cache = {}


def _build_nc():
    from contextlib import ExitStack

    import concourse.tile as tile
    from concourse import bacc, mybir

    f32 = mybir.dt.float32
    f32r = mybir.dt.float32r
    Tanh = mybir.ActivationFunctionType.Tanh
    Exp = mybir.ActivationFunctionType.Exp

    nc = bacc.Bacc(
        "TRN2",
        target_bir_lowering=False,
        debug=False,
        enable_asserts=False,
        num_devices=NCORES,
    )

    keysT = nc.dram_tensor("keysT", [D, BL, S], f32r, kind="ExternalInput").ap()
    w2t = nc.dram_tensor("w2t", [128, KB, A], f32r, kind="ExternalInput").ap()
    w1t = nc.dram_tensor("w1t", [128, KB, A], f32r, kind="ExternalInput").ap()
    qT = nc.dram_tensor("qT", [128, KB, BL], f32r, kind="ExternalInput").ap()
    # vz[p, j, b, c] = v[j*128+p] if b == c else 0 — one-hot column per batch so
    # each batch's v-dot lands in its own psum row
    vz = nc.dram_tensor("vz", [128, JB * BL * BL], f32r, kind="ExternalInput").ap()
    maskadd = nc.dram_tensor("maskadd", [BL, S], f32, kind="ExternalInput").ap()
    out = nc.dram_tensor("out", [BL, S], f32, kind="ExternalOutput").ap()

    keysT_r = keysT.rearrange("(k p) b s -> p k b s", p=128)

    with tile.TileContext(nc) as tc, ExitStack() as ctx:
        singles = ctx.enter_context(tc.tile_pool(name="singles", bufs=1))
        keysp = ctx.enter_context(tc.tile_pool(name="keys", bufs=3))
        combp = ctx.enter_context(tc.tile_pool(name="comb", bufs=4))
        psmain = ctx.enter_context(tc.tile_pool(name="psmain", bufs=2, space="PSUM"))
        psvdot = ctx.enter_context(tc.tile_pool(name="psvdot", bufs=2, space="PSUM"))
        psw1q = ctx.enter_context(tc.tile_pool(name="psw1q", bufs=2, space="PSUM"))

        w2_sb = singles.tile([128, KB, A], f32r)
        nc.sync.dma_start(w2_sb[:], w2t)
        w1_sb = singles.tile([128, KB, A], f32r)
        nc.sync.dma_start(w1_sb[:], w1t)
        q_sb = singles.tile([128, KB, BL], f32r)
        nc.sync.dma_start(q_sb[:], qT)
        v_sb = singles.tile([128, JB * BL * BL], f32r)
        nc.sync.dma_start(v_sb[:], vz)
        ma_sb = singles.tile([BL, S], f32)
        nc.sync.dma_start(ma_sb[:], maskadd)
        scores = singles.tile([BL, S], f32)
        w1q = singles.tile([128, JB, BL], f32)

        # w1q[a, b] = sum_d W1[a, d] q[b, d], a on partitions
        for j in range(JB):
            wq_ps = psw1q.tile([128, BL], f32)
            for k in range(KB):
                nc.tensor.matmul(
                    wq_ps[:],
                    lhsT=w1_sb[:, k, j * 128 : (j + 1) * 128],
                    rhs=q_sb[:, k, :],
                    start=(k == 0),
                    stop=(k == KB - 1),
                )
            nc.scalar.copy(w1q[:, j, :], wq_ps[:])

        # main loop; vdot matmuls are emitted one j-slot late so the PE never
        # waits on the tanh of the block it just produced
        pending = []

        def emit_vdot(args):
            sc_ps, j, comb, b, st = args
            nc.tensor.matmul(
                sc_ps[:],
                lhsT=v_sb[:, (j * BL + b) * BL : (j * BL + b + 1) * BL],
                rhs=comb[:],
                start=(b == 0 and j == 0),
                stop=(b == BL - 1 and j == JB - 1),
            )
            if b == BL - 1 and j == JB - 1:
                # all 4 batch rows of this s-tile = v-dot + additive mask
                nc.vector.tensor_add(
                    scores[:, st * ST : (st + 1) * ST],
                    sc_ps[:, :],
                    ma_sb[:, st * ST : (st + 1) * ST],
                )

        for st in range(NST):
            sc_ps = psvdot.tile([BL, ST], f32)
            for b in range(BL):
                kt = keysp.tile([128, KB, ST], f32r)
                nc.sync.dma_start(kt[:], keysT_r[:, :, b, st * ST : (st + 1) * ST])
                for j in range(JB):
                    ps = psmain.tile([128, ST], f32)
                    for k in range(KB):
                        nc.tensor.matmul(
                            ps[:],
                            lhsT=w2_sb[:, k, j * 128 : (j + 1) * 128],
                            rhs=kt[:, k, :],
                            start=(k == 0),
                            stop=(k == KB - 1),
                        )
                    comb = combp.tile([128, ST], f32r)
                    nc.scalar.activation(
                        comb[:], ps[:], Tanh, bias=w1q[:, j, b : b + 1]
                    )
                    pending.append((sc_ps, j, comb, b, st))
                    if len(pending) > 1:
                        emit_vdot(pending.pop(0))
        while pending:
            emit_vdot(pending.pop(0))

        # masked softmax over S for the 4 batch rows
        nmx = singles.tile([BL, 1], f32)
        nc.vector.tensor_reduce(
            nmx[:],
            scores[:],
            axis=mybir.AxisListType.X,
            op=mybir.AluOpType.max,
            negate=True,
        )
        e_sb = singles.tile([BL, S], f32)
        sm = singles.tile([BL, 1], f32)
        nc.scalar.activation(e_sb[:], scores[:], Exp, bias=nmx[:, 0:1], accum_out=sm[:])
        rs = singles.tile([BL, 1], f32)
        nc.vector.reciprocal(rs[:], sm[:])
        o_sb = singles.tile([BL, S], f32)
        nc.vector.tensor_scalar_mul(o_sb[:], e_sb[:], rs[:, 0:1])
        nc.sync.dma_start(out, o_sb[:])

    nc.compile()
    return nc


def _get_nc():
    if "nc" not in _cache:
        _cache["nc"] = _build_nc()
    return _cache["nc"]


def _make_in_maps(query, keys, mask, W1, W2, v):
    query = np.asarray(query, dtype=np.float32)
    keys = np.asarray(keys, dtype=np.float32)
    mask = np.asarray(mask)
    W1 = np.asarray(W1, dtype=np.float32)
    W2 = np.asarray(W2, dtype=np.float32)
    v = np.asarray(v, dtype=np.float32)

    # replicated weight layouts
    w2t = np.ascontiguousarray(W2.T.reshape(KB, 128, A).transpose(1, 0, 2))
    w1t = np.ascontiguousarray(W1.T.reshape(KB, 128, A).transpose(1, 0, 2))
    vz = np.zeros((128, JB, BL, BL), dtype=np.float32)
    vcols = v.reshape(JB, 128).T  # [p, j]
    for b in range(BL):
        vz[:, :, b, b] = vcols
    vz = np.ascontiguousarray(vz.reshape(128, JB * BL * BL))

    in_maps = []
    for c in range(NCORES):
        sl = slice(c * BL, (c + 1) * BL)
        keysT_c = np.ascontiguousarray(keys[sl].transpose(2, 0, 1))  # (D, BL, S)
        qT_c = np.ascontiguousarray(
            query[sl].T.reshape(KB, 128, BL).transpose(1, 0, 2)
        )  # (128, KB, BL)
        maskadd_c = np.where(mask[sl], 0.0, -MASK_NEG).astype(np.float32)
        in_maps.append(
            {
                "keysT": keysT_c,
                "w2t": w2t,
                "w1t": w1t,
                "qT": qT_c,
                "vz": vz,
                "maskadd": maskadd_c,
            }
        )
    return in_maps


def kernel(query, keys, mask, W1, W2, v):
    from concourse.bass_utils import run_bass_kernel_spmd

    nc = _get_nc()
    in_maps = _make_in_maps(query, keys, mask, W1, W2, v)
    res = run_bass_kernel_spmd(nc, in_maps, core_ids=list(range(NCORES)))
    _cache["last_results"] = res
    out = np.concatenate([res.results[i]["out"] for i in range(NCORES)], axis=0)
    return out.astype(np.float32)


# revision 9
# speedup vs baseline: 1.0879x; 1.0879x over previous
"""Additive (Bahdanau) attention kernel for one TRN2 chip (8 NeuronCores).

Computes, for query (B,D), keys (B,S,D), mask (B,S), W1 (A,D), W2 (A,D), v (A,):
    scores[b,s] = v . tanh(W1 @ query[b] + W2 @ keys[b,s])
    out = softmax(scores - 1e30 * ~mask, axis=-1)

Sharding: data-parallel over batch B across the 8 cores (4 batches/core);
W1/W2/v replicated. No collectives needed; per-core outputs are concatenated
on the host.

Per-core device kernel (main matmuls in float32r at full PE rate):
  - w1q[a,b]    = W1 @ q_b              (tiny matmul, a on partitions)
  - per (s-tile of 512, b), per a-block j of 128:
        psum[a,s] += W2T_blk.T @ keysT_tile    (8 k-blocks, PE)
        comb = tanh(psum + w1q[:,b])           (ScalarE, per-partition bias)
        acc  += v_j * comb                     (VectorE mul+add chain)
    last add writes acc in f32r; a one-hot ones matmul per (s-tile, b)
    partition-reduces acc into row b of a shared [4, 512] psum tile
  - scores[:, s-tile] = sc_psum + maskadd      (additive -1e30 mask)
  - softmax tail on [4, 2048]: -max -> exp(+bias, accum sum) -> recip -> scale

DMA order is staged (q, W1/W2 a-block j=0, first keys tile, then remaining
a-blocks per j) so the PE starts ~8us in instead of waiting for 10 MB.
"""

import numpy as np

B, S, D, A = 32, 2048, 1024, 1024
NCORES = 8
BL = B // NCORES  # 4 batches per core
ST = 512          # s-tile width
NST = S // ST     # 4 s-tiles per batch
KB = D // 128     # 8 contraction blocks
JB = A // 128     # 8 attn-dim blocks
MASK_NEG = 1e30

_cache = {}


def _build_nc():
    from contextlib import ExitStack

    import concourse.tile as tile
    from concourse import bacc, mybir

    f32 = mybir.dt.float32
    f32r = mybir.dt.float32r
    Tanh = mybir.ActivationFunctionType.Tanh
    Exp = mybir.ActivationFunctionType.Exp

    nc = bacc.Bacc(
        "TRN2",
        target_bir_lowering=False,
        debug=False,
        enable_asserts=False,
        num_devices=NCORES,
    )

    keysT = nc.dram_tensor("keysT", [D, BL, S], f32r, kind="ExternalInput").ap()
    w2t = nc.dram_tensor("w2t", [128, KB, A], f32r, kind="ExternalInput").ap()
    w1t = nc.dram_tensor("w1t", [128, KB, A], f32r, kind="ExternalInput").ap()
    qT = nc.dram_tensor("qT", [128, KB, BL], f32r, kind="ExternalInput").ap()
    # vcol[p, j] = v[j*128+p] — per-partition scalar for the DVE multiply
    vcol = nc.dram_tensor("vcol", [128, JB], f32, kind="ExternalInput").ap()
    # onesz[p, b, c] = 1 if b == c else 0 — one-hot ones column per batch so
    # each batch's partition-reduce lands in its own psum row
    onesz = nc.dram_tensor("onesz", [128, BL * BL], f32r, kind="ExternalInput").ap()
    maskadd = nc.dram_tensor("maskadd", [BL, S], f32, kind="ExternalInput").ap()
    out = nc.dram_tensor("out", [BL, S], f32, kind="ExternalOutput").ap()

    keysT_r = keysT.rearrange("(k p) b s -> p k b s", p=128)

    with tile.TileContext(nc) as tc, ExitStack() as ctx:
        singles = ctx.enter_context(tc.tile_pool(name="singles", bufs=1))
        keysp = ctx.enter_context(tc.tile_pool(name="keys", bufs=2))
        combp = ctx.enter_context(tc.tile_pool(name="comb", bufs=3))
        accp = ctx.enter_context(tc.tile_pool(name="acc", bufs=3))
        accrp = ctx.enter_context(tc.tile_pool(name="accr", bufs=2))
        tmpp = ctx.enter_context(tc.tile_pool(name="tmp", bufs=3))
        psmain = ctx.enter_context(tc.tile_pool(name="psmain", bufs=2, space="PSUM"))
        psvdot = ctx.enter_context(tc.tile_pool(name="psvdot", bufs=2, space="PSUM"))
        psw1q = ctx.enter_context(tc.tile_pool(name="psw1q", bufs=2, space="PSUM"))

        # --- staged input DMAs ---------------------------------------------
        q_sb = singles.tile([128, KB, BL], f32r)
        nc.sync.dma_start(q_sb[:], qT)
        v_sb = singles.tile([128, JB], f32)
        nc.sync.dma_start(v_sb[:], vcol)
        o_one = singles.tile([128, BL * BL], f32r)
        nc.sync.dma_start(o_one[:], onesz)
        ma_sb = singles.tile([BL, S], f32)
        nc.sync.dma_start(ma_sb[:], maskadd)

        w1_sb = singles.tile([128, KB, A], f32r)
        w2_sb = singles.tile([128, KB, A], f32r)
        # j=0 columns of W1 and W2 first, then the first keys tile, then the
        # rest of the weight columns interleaved per j
        nc.sync.dma_start(w1_sb[:, :, 0:128], w1t[:, :, 0:128])
        nc.sync.dma_start(w2_sb[:, :, 0:128], w2t[:, :, 0:128])
        kt0 = keysp.tile([128, KB, ST], f32r)
        nc.sync.dma_start(kt0[:], keysT_r[:, :, 0, 0:ST])
        for j in range(1, JB):
            sl = slice(j * 128, (j + 1) * 128)
            nc.sync.dma_start(w1_sb[:, :, sl], w1t[:, :, sl])
            nc.sync.dma_start(w2_sb[:, :, sl], w2t[:, :, sl])

        scores = singles.tile([BL, S], f32)
        w1q = singles.tile([128, JB, BL], f32)

        # w1q[a, b] = sum_d W1[a, d] q[b, d], a on partitions
        for j in range(JB):
            wq_ps = psw1q.tile([128, BL], f32)
            for k in range(KB):
                nc.tensor.matmul(
                    wq_ps[:],
                    lhsT=w1_sb[:, k, j * 128 : (j + 1) * 128],
                    rhs=q_sb[:, k, :],
                    start=(k == 0),
                    stop=(k == KB - 1),
                )
            nc.scalar.copy(w1q[:, j, :], wq_ps[:])

        # --- main loop ------------------------------------------------------
        for st in range(NST):
            sc_ps = psvdot.tile([BL, ST], f32)
            for b in range(BL):
                if st == 0 and b == 0:
                    kt = kt0
                else:
                    kt = keysp.tile([128, KB, ST], f32r)
                    nc.sync.dma_start(
                        kt[:], keysT_r[:, :, b, st * ST : (st + 1) * ST]
                    )
                acc = accp.tile([128, ST], f32)
                accr = accrp.tile([128, ST], f32r)
                for j in range(JB):
                    ps = psmain.tile([128, ST], f32)
                    for k in range(KB):
                        nc.tensor.matmul(
                            ps[:],
                            lhsT=w2_sb[:, k, j * 128 : (j + 1) * 128],
                            rhs=kt[:, k, :],
                            start=(k == 0),
                            stop=(k == KB - 1),
                        )
                    comb = combp.tile([128, ST], f32)
                    nc.scalar.activation(
                        comb[:], ps[:], Tanh, bias=w1q[:, j, b : b + 1]
                    )
                    # acc += v_j * comb on VectorE
                    if j == 0:
                        nc.vector.tensor_scalar_mul(acc[:], comb[:], v_sb[:, 0:1])
                    else:
                        tmp = tmpp.tile([128, ST], f32)
                        nc.vector.tensor_scalar_mul(
                            tmp[:], comb[:], v_sb[:, j : j + 1]
                        )
                        if j == JB - 1:
                            nc.vector.tensor_add(accr[:], acc[:], tmp[:])
                        else:
                            nc.vector.tensor_add(acc[:], acc[:], tmp[:])
                # partition-reduce acc into row b of sc_ps
                nc.tensor.matmul(
                    sc_ps[:],
                    lhsT=o_one[:, b * BL : (b + 1) * BL],
                    rhs=accr[:],
                    start=(b == 0),
                    stop=(b == BL - 1),
                )
            nc.vector.tensor_add(
                scores[:, st * ST : (st + 1) * ST],
                sc_ps[:, :],
                ma_sb[:, st * ST : (st + 1) * ST],
            )

        # --- masked softmax over S for the 4 batch rows ---------------------
        nmx = singles.tile([BL, 1], f32)
        nc.vector.tensor_reduce(
            nmx[:],
            scores[:],
            axis=mybir.AxisListType.X,
            op=mybir.AluOpType.max,
            negate=True,
        )
        e_sb = singles.tile([BL, S], f32)
        sm = singles.tile([BL, 1], f32)
        nc.scalar.activation(e_sb[:], scores[:], Exp, bias=nmx[:, 0:1], accum_out=sm[:])
        rs = singles.tile([BL, 1], f32)
        nc.vector.reciprocal(rs[:], sm[:])
        o_sb = singles.tile([BL, S], f32)
        nc.vector.tensor_scalar_mul(o_sb[:], e_sb[:], rs[:, 0:1])
        nc.sync.dma_start(out, o_sb[:])

    nc.compile()
    return nc


def _get_nc():
    if "nc" not in _cache:
        _cache["nc"] = _build_nc()
    return _cache["nc"]


def _make_in_maps(query, keys, mask, W1, W2, v):
    query = np.asarray(query, dtype=np.float32)
    keys = np.asarray(keys, dtype=np.float32)
    mask = np.asarray(mask)
    W1 = np.asarray(W1, dtype=np.float32)
    W2 = np.asarray(W2, dtype=np.float32)
    v = np.asarray(v, dtype=np.float32)

    # replicated weight layouts
    w2t = np.ascontiguousarray(W2.T.reshape(KB, 128, A).transpose(1, 0, 2))
    w1t = np.ascontiguousarray(W1.T.reshape(KB, 128, A).transpose(1, 0, 2))
    vcol = np.ascontiguousarray(v.reshape(JB, 128).T)  # [p, j]
    onesz = np.zeros((128, BL, BL), dtype=np.float32)
    for b in range(BL):
        onesz[:, b, b] = 1.0
    onesz = np.ascontiguousarray(onesz.reshape(128, BL * BL))

    in_maps = []
    for c in range(NCORES):
        sl = slice(c * BL, (c + 1) * BL)
        keysT_c = np.ascontiguousarray(keys[sl].transpose(2, 0, 1))  # (D, BL, S)
        qT_c = np.ascontiguousarray(
            query[sl].T.reshape(KB, 128, BL).transpose(1, 0, 2)
        )  # (128, KB, BL)
        maskadd_c = np.where(mask[sl], 0.0, -MASK_NEG).astype(np.float32)
        in_maps.append(
            {
                "keysT": keysT_c,
                "w2t": w2t,
                "w1t": w1t,
                "qT": qT_c,
                "vcol": vcol,
                "onesz": onesz,
                "maskadd": maskadd_c,
            }
        )
    return in_maps


def kernel(query, keys, mask, W1, W2, v):
    from concourse.bass_utils import run_bass_kernel_spmd

    nc = _get_nc()
    in_maps = _make_in_maps(query, keys, mask, W1, W2, v)
    res = run_bass_kernel_spmd(nc, in_maps, core_ids=list(range(NCORES)))
    _cache["last_results"] = res
    out = np.concatenate([res.results[i]["out"] for i in range(NCORES)], axis=0)
    return out.astype(np.float32)
